# revision 1
# baseline (speedup 1.0000x reference)
"""Trainium2 Bass kernel for nn_DiffusionModule (self-similarity diffusion).

Math (per batch b, with src = feature_src[b].reshape(C, N)):
    P   = src^T @ src                      # [N, N], sim = P / sqrt(C)
    Pbar_n = mean_m P[m, n]  (= 16*mu_n; P symmetric)
    aff[n, m] = exp(-((P[n,m] - Pbar_n) / (16*sqrt(2)))^2)   # sigma=1, C=256
    D = aff / rowsum(aff)
    out = 0.5 * (src @ D^T) + 0.5 * dst

Sharding: 8 cores = 4 batches x 2 column-halves (rows n of the affinity).
Each core computes out[:, R] for its 2048-column block R, SPMD (identical
program, per-core data): full `src`, its column slice `srcR`, dst slice.

Kernel layout (per core):
  - everything in "transposed" [m partitions, n free] layout so no big
    transposes of the affinity matrix are ever needed:
      * simT psum block [128m, 512n] = 2 K-chunk matmuls (float32r, full rate)
        + a rank-1 (K=1, ones x (-Pbar/4096)) matmul that subtracts the mean
      * ScalarE: Square (scaled) then Exp -> affT tile in SBUF
      * 2nd matmul uses affT chunks as lhsT (K=m) and srcT (with an appended
        ones column -> free row-sums!) as rhs -> out2 psum [128n, 257]
      * per-partition (n) normalize on DVE, PE transpose back to [c, n],
        blend with 0.5*dst, DMA out.
"""

import os
import threading

import numpy as np

_KERNEL_CACHE = {}
_LOCK = threading.Lock()

B, C, H, W = 4, 256, 64, 64
N = H * W * H // 64  # 4096 = H*W
N = 4096
HALF = N // 2  # columns per core
NBLK = 512  # n-block width
N_NBLK = HALF // NBLK  # 4
MT = N // 128  # 32 m-tiles
KC = C // 128  # 2 contraction chunks
SCL = 1.0 / (16.0 * np.sqrt(2.0))  # (P-Pbar)*SCL squared == (sim-mu)^2/2
ALPHA = 0.5
EPS = 1e-12


def _build():
    """Build + compile the SPMD Bass program once. Returns (nc, names)."""
    from contextlib import ExitStack

    import concourse.bass as bass
    import concourse.tile as tile
    from concourse import bacc, mybir
    from concourse.masks import make_identity

    fp32 = mybir.dt.float32
    bf16 = mybir.dt.bfloat16
    use_f32r = os.environ.get("KERNEL_EXACT_FP32", "0") != "1"
    # R = dtype of the matmul datapath (float32r = full-rate PE, fp32 bits)
    R = mybir.dt.float32r if use_f32r else fp32

    nc = bacc.Bacc(
        "TRN2", target_bir_lowering=False, debug=False, num_devices=8
    )

    src_d = nc.dram_tensor("src", [C, N], R, kind="ExternalInput").ap()
    srcr_d = nc.dram_tensor("srcr", [C, HALF], R, kind="ExternalInput").ap()
    dst_d = nc.dram_tensor("dst", [C, HALF], fp32, kind="ExternalInput").ap()
    out_d = nc.dram_tensor("out", [C, HALF], fp32, kind="ExternalOutput").ap()

    reps = int(os.environ.get("KERNEL_REPS", "1"))

    with tile.TileContext(nc) as tc, ExitStack() as ctx:
        singles = ctx.enter_context(tc.tile_pool(name="singles", bufs=1))
        # PSUM pools: "ps" (2-bank slots x2) for sim groups / transposes,
        # "o" (1-bank slots x4) for out2 accumulators + meanP rows.
        pspool = ctx.enter_context(tc.tile_pool(name="ps", bufs=2, space="PSUM"))
        opool = ctx.enter_context(tc.tile_pool(name="o", bufs=4, space="PSUM"))
        y2pool = ctx.enter_context(tc.tile_pool(name="y2", bufs=2))
        affpool = ctx.enter_context(tc.tile_pool(name="aff", bufs=3))
        outpool = ctx.enter_context(tc.tile_pool(name="outsb", bufs=4))
        smallp = ctx.enter_context(tc.tile_pool(name="small", bufs=8))

        for _rep in range(reps):
            # ---------------- stage 0: loads + prep ----------------
            sb_src = singles.tile([128, KC, N], R)
            nc.sync.dma_start(sb_src, src_d.rearrange("(k p) n -> p k n", p=128))
            sb_srcr = singles.tile([128, KC, HALF], R)
            nc.sync.dma_start(sb_srcr, srcr_d.rearrange("(k p) n -> p k n", p=128))
            sb_dsth = singles.tile([128, KC, HALF], fp32)
            nc.sync.dma_start(sb_dsth, dst_d.rearrange("(k p) n -> p k n", p=128))
            # dsth = 0.5*dst in place (2x_2P fp32 tensor_scalar on SBUF)
            for k in range(KC):
                nc.vector.tensor_scalar_mul(
                    sb_dsth[:, k, :], sb_dsth[:, k, :], ALPHA
                )

            identity32 = singles.tile([128, 128], fp32)
            make_identity(nc, identity32)
            ones32 = singles.tile([1, 128], fp32)
            nc.vector.memset(ones32, 1.0)
            ones_row = singles.tile([1, 128], R)
            nc.vector.tensor_copy(ones_row, ones32)

            col32 = singles.tile([128, 1], fp32)
            nc.vector.memset(col32, 1.0)

            # srcT [m, c] bf16 with appended ones column (row-sum trick)
            sb_srcT = singles.tile([128, MT, C + 1], bf16)
            for mt in range(MT):
                nc.vector.tensor_copy(sb_srcT[:, mt, C : C + 1], col32)
                ps_t = pspool.tile([128, 256], fp32, tag="g", name="ps_t")
                for k in range(KC):
                    nc.tensor.transpose(
                        ps_t[:, k * 128 : (k + 1) * 128],
                        sb_src[:, k, mt * 128 : (mt + 1) * 128].bitcast(fp32),
                        identity32,
                    )
                nc.vector.tensor_copy(sb_srcT[:, mt, 0:C], ps_t)

            # s = per-channel row-sums of src -> [128, KC]
            s_col = singles.tile([128, KC], fp32)
            for k in range(KC):
                nc.vector.reduce_sum(
                    s_col[:, k : k + 1],
                    sb_src[:, k, :].bitcast(fp32),
                    axis=mybir.AxisListType.X,
                )

            # negmurow[0, n] = -Pbar_n / ... = -(sum_m P[m,n]) / N for n in R
            negmu32 = singles.tile([1, HALF], fp32)
            negmu = singles.tile([1, HALF], R)
            for nb in range(N_NBLK):
                ps_mp = opool.tile([128, 512], fp32, tag="o", name="ps_mp")
                for k in range(KC):
                    nc.tensor.matmul(
                        ps_mp[0:1, 0:NBLK],
                        s_col[:, k : k + 1],
                        sb_srcr[:, k, nb * NBLK : (nb + 1) * NBLK].bitcast(fp32),
                        start=(k == 0),
                        stop=(k == KC - 1),
                    )
                nc.vector.tensor_scalar_mul(
                    negmu32[:, nb * NBLK : (nb + 1) * NBLK],
                    ps_mp[0:1, 0:NBLK],
                    -1.0 / float(N),
                )
            nc.vector.tensor_copy(negmu, negmu32)

            # ---------------- main loop over n-blocks ----------------
            # Software-pipelined emission: each group's 2nd-matmul chunks are
            # deferred one group (PE runs next group's sim MMs while ACT does
            # Square/Exp), and each n-block's out-stage is deferred into the
            # next n-block's first group.
            pending_out = None  # thunk for previous n-block's out stage

            def emit_out_stage(po, n0):
                # normalize rows (n on partitions), transpose back, blend, DMA
                outT = []
                for q in range(4):
                    sq = smallp.tile([128, 1], fp32, name="sq")
                    # sq = 1 / (2*max(rowsum, EPS))  == alpha/denom
                    nc.vector.tensor_scalar(
                        sq,
                        po[q][:, C : C + 1],
                        EPS,
                        1.0 / ALPHA,
                        op0=mybir.AluOpType.max,
                        op1=mybir.AluOpType.mult,
                    )
                    nc.vector.reciprocal(sq, sq)
                    ot = outpool.tile([128, C], fp32, tag="outT", name="outT")
                    nc.vector.tensor_scalar_mul(ot, po[q][:, 0:C], sq)
                    outT.append(ot)
                for cb in range(KC):
                    ps_tb = pspool.tile([128, NBLK], fp32, tag="g", name="ps_tb")
                    for q in range(4):
                        nc.tensor.transpose(
                            ps_tb[:, q * 128 : (q + 1) * 128],
                            outT[q][:, cb * 128 : (cb + 1) * 128],
                            identity32,
                        )
                    ob = outpool.tile([128, NBLK], fp32, tag="ob", name="ob")
                    nc.vector.tensor_add(
                        ob, ps_tb, sb_dsth[:, cb, n0 : n0 + NBLK]
                    )
                    nc.sync.dma_start(
                        out_d[cb * 128 : (cb + 1) * 128, n0 : n0 + NBLK], ob
                    )

            for nb in range(N_NBLK):
                n0 = nb * NBLK
                # out2 accumulators: 4 partition-chunks of n, [128n, 257]
                po = [
                    opool.tile([128, 512], fp32, tag="o", name=f"po{q}")
                    for q in range(4)
                ]
                ps_sim = None
                y2 = None
                # deferred 2nd-matmul chunks: list of (afft, g4, j)
                mm2_q = []

                def emit_mm2_chunk():
                    afft_, g4_, j_ = mm2_q.pop(0)
                    mtg = g4_ * 4 + j_
                    for q in range(4):
                        nc.tensor.matmul(
                            po[q][:, 0 : C + 1],
                            afft_[:, j_, q * 128 : (q + 1) * 128],
                            sb_srcT[:, mtg, :],
                            start=(mtg == 0),
                            stop=(mtg == MT - 1),
                        )

                for mt in range(MT):
                    gi = mt % 2
                    if gi == 0:
                        ps_sim = pspool.tile(
                            [128, 2, NBLK], fp32, tag="g", name="ps_sim"
                        )
                    psv = ps_sim[:, gi, :]
                    for k in range(KC):
                        nc.tensor.matmul(
                            psv,
                            sb_src[:, k, mt * 128 : (mt + 1) * 128],
                            sb_srcr[:, k, n0 : n0 + NBLK],
                            start=(k == 0),
                            stop=False,
                        )
                    nc.tensor.matmul(
                        psv,
                        ones_row,
                        negmu[:, n0 : n0 + NBLK],
                        start=False,
                        stop=True,
                    )
                    if mm2_q:
                        emit_mm2_chunk()
                    if mt % 4 == 0:
                        y2 = y2pool.tile([128, 4, NBLK], fp32, name="y2")
                    if gi == 1:
                        # Square the 2-tile group: y2 half <- (psum*SCL)^2
                        half = (mt % 4) // 2
                        nc.scalar.activation(
                            y2[:, half * 2 : half * 2 + 2, :],
                            ps_sim[:, :, :],
                            mybir.ActivationFunctionType.Square,
                            scale=SCL,
                        )
                    if mt % 4 == 3:
                        g4 = mt // 4  # group of 4 m-tiles
                        afft = affpool.tile([128, 4, NBLK], bf16, name="afft")
                        nc.scalar.activation(
                            afft[:, :, :],
                            y2[:, :, :],
                            mybir.ActivationFunctionType.Exp,
                            scale=-1.0,
                        )
                        for j in range(4):
                            mm2_q.append((afft, g4, j))
                    if mt == 3 and pending_out is not None:
                        pending_out()
                        pending_out = None
                while mm2_q:
                    emit_mm2_chunk()
                pending_out = (lambda po=po, n0=n0: emit_out_stage(po, n0))
            pending_out()
            pending_out = None

    nc.compile()
    return nc


def _get_compiled():
    with _LOCK:
        key = (
            os.environ.get("KERNEL_EXACT_FP32", "0"),
            os.environ.get("KERNEL_REPS", "1"),
        )
        if key not in _KERNEL_CACHE:
            _KERNEL_CACHE[key] = _build()
        return _KERNEL_CACHE[key]


def _make_in_maps(feature_src, feature_dst):
    src = np.ascontiguousarray(
        np.asarray(feature_src, dtype=np.float32).reshape(B, C, N)
    )
    dst = np.ascontiguousarray(
        np.asarray(feature_dst, dtype=np.float32).reshape(B, C, N)
    )
    in_maps = []
    for core in range(8):
        b, h = core // 2, core % 2
        sl = slice(h * HALF, (h + 1) * HALF)
        in_maps.append(
            {
                "src": src[b],
                "srcr": np.ascontiguousarray(src[b][:, sl]),
                "dst": np.ascontiguousarray(dst[b][:, sl]),
            }
        )
    return in_maps


def _assemble(results):
    out = np.empty((B, C, N), dtype=np.float32)
    for core in range(8):
        b, h = core // 2, core % 2
        out[b][:, h * HALF : (h + 1) * HALF] = results[core]["out"]
    return out.reshape(B, C, H, W)


def run(feature_src, feature_dst, trace=False):
    """Run on 8 NeuronCores; returns (output [B,C,H,W], exec_time_ns|None)."""
    from concourse import bass_utils

    nc = _get_compiled()
    in_maps = _make_in_maps(feature_src, feature_dst)
    res = bass_utils.run_bass_kernel_spmd(
        nc, in_maps, core_ids=list(range(8)), trace=trace
    )
    return _assemble(res.results), res.exec_time_ns


def kernel(feature_src, feature_dst):
    out, _ = run(feature_src, feature_dst, trace=False)
    return out

